# revision 21
# baseline (speedup 1.0000x reference)
"""Trainium2 Bass kernel for the directional min-variance filter (Kuwahara-style).

Algorithm (per image, fp32):
  For each of 8 directions d (rays of 8 pixels from each pixel):
    x1_d = directional sum of x, y2_d = directional sum of x^2
    metric m'_d = x1_d^2/8 - y2_d          (= -(y2-y1); maximize m' == minimize var)
  out = x1_{argmax m'} / 8   with first-index-wins tie semantics (matches argmin).

Layout ("all free-dim"): the 1024x1024 image is split into 128 blocks of
64 rows x 128 cols; partition p = cb*16 + rc owns block (rc, cb) and stores
it row-major with a 7-pixel halo on every side: 78 rows x 142 cols.  Every
directional shift is then a pure free-dim offset, so all sums run on the
vector engine with plain 2-operand adds (log2 doubling: 3 adds per 8-sum).
Forward/backward direction pairs share sums: the backward ray sum at (y,x)
equals the forward sum at (y,x) - 7*u, read as an offset view.

8 NeuronCores run pure data-parallel over the batch of 8 images.
"""

import numpy as np

import concourse.bass as bass
import concourse.bacc as bacc
import concourse.tile as tile
from concourse import mybir
from concourse.bass_utils import run_bass_kernel_spmd

F32 = mybir.dt.float32

H = W = 1024
A = 64          # rows per block
B = 128         # cols per block
NRC = 16        # row-chunks  (rc)
NCB = 8         # col-blocks  (cb)
PITCH = B + 14  # 142
XR = A + 14     # 78 stored rows
S = 8           # slab rows
NS = A // S     # 8 slabs

PAD = 16
SZ_X = XR * PITCH              # 11076
SZ_XSQ = (S + 14) * PITCH      # 3124
SZ_S1 = (S + 13) * PITCH       # 2982
SZ_S2 = (S + 11) * PITCH       # 2698
SZ_F = (S + 7) * PITCH         # 2130
SZ_O = S * PITCH               # 1136 (pitch-142 so views stay 3D like m/x1 views)

OFF_X = 0
OFF_XSQ = OFF_X + SZ_X + PAD
OFF_S1 = OFF_XSQ + SZ_XSQ + PAD
OFF_S2 = OFF_S1 + SZ_S1 + PAD
OFF_F = OFF_S2 + SZ_S2 + PAD                    # 4 slots (x1 per fwd dir)
OFF_Y2 = OFF_F + 4 * (SZ_F + PAD)               # 2 rotating slots
OFF_M = OFF_Y2 + 2 * (SZ_F + PAD)               # 4 slots (metric per fwd dir)
OFF_BM = OFF_M + 4 * (SZ_F + PAD)
OFF_BX = OFF_BM + SZ_O + PAD
OFF_OUT = OFF_BX + SZ_O + PAD                   # 2 rotating slots
OFF_TOUCH = OFF_OUT + 2 * (SZ_O + PAD)          # fan-in scratch (DVE, ACT)
TOTAL = OFF_TOUCH + 2 * (PITCH + PAD) + PITCH

# forward dirs: name -> (uy, ux, F-slot index)
FWD = {"a": (0, 1, 0), "b": (1, 0, 1), "c": (1, 1, 2), "e": (1, -1, 3)}
# chain in reference dir order: (fwd-name, dy, dx) where the d-th direction's
# metric/payload = fwd buffer read at offset (dy, dx)
CHAIN = [
    ("c", -7, -7),  # d0 (-1,-1)
    ("b", -7, 0),   # d1 (-1, 0)
    ("e", -7, 7),   # d2 (-1, 1)
    ("a", 0, -7),   # d3 (0,-1)
    ("a", 0, 0),    # d4 (0, 1)
    ("e", 0, 0),    # d5 (1,-1)
    ("b", 0, 0),    # d6 (1, 0)
    ("c", 0, 0),    # d7 (1, 1)
]


XPR = H + 14    # padded scratch image rows
XPC = 1040      # padded scratch image cols (1024 + 14, rounded up)


def _build():
    nc = bacc.Bacc("TRN2", target_bir_lowering=False)
    x_t = nc.declare_dram_parameter("x", [H, W], F32, isOutput=False)
    y_t = nc.declare_dram_parameter("y", [H, W], F32, isOutput=True)
    xp_t = nc.dram_tensor("xp", [XPR, XPC], F32, kind="Internal")
    x_ap = x_t[:]
    y_ap = y_t[:]
    xp_ap = xp_t[:]

    with tile.TileContext(nc) as tc:
        with tc.tile_pool(name="main", bufs=1) as pool:
            big = pool.tile([128, TOTAL], F32)
            lt8 = pool.tile([128, S * PITCH + PAD], mybir.dt.uint8)

            def view(off, r0, r1, c0, c1, p0=0, p1=128):
                # [p1-p0, r1-r0, c1-c0] view of a pitch-PITCH buffer at `off`
                start = off + r0 * PITCH + c0
                ln = (r1 - r0) * PITCH
                return big[p0:p1, start:start + ln].rearrange(
                    "p (r c) -> p r c", c=PITCH)[:, :, 0:c1 - c0]

            def xv(y0, y1, x0, x1):
                # X view in output coords (origin row -7, col -7)
                return view(OFF_X, y0 + 7, y1 + 7, x0 + 7, x1 + 7)

            def sv(off, s0, y0, y1, x0, x1):
                # slab-local buffer view, buffer origin at (s0-7, -7)
                return view(off, y0 - (s0 - 7), y1 - (s0 - 7), x0 + 7, x1 + 7)

            def o3(off, p0=0, p1=128):
                # output-region view [P, S, B] on a pitch-PITCH buffer
                return view(off, 0, S, 0, B, p0, p1)

            # ---------------- input load ----------------
            # Stage a zero-padded copy of the image in DRAM scratch so the
            # SBUF load is fully uniform per partition (no border clipping).
            # xp[r, c] = img[r - 7, c - 7], zero outside.
            # zero all of xp by broadcasting an inline DRAM row of zeros
            zrow = nc.inline_tensor(np.zeros((1, XPC), np.float32), name="zrow")
            nc.sync.dma_start(
                out=bass.AP(tensor=xp_ap.tensor, offset=xp_ap.offset,
                            ap=[[XPC, XPR], [1, XPC]]),
                in_=bass.AP(tensor=zrow[:].tensor, offset=zrow[:].offset,
                            ap=[[0, XPR], [1, XPC]]))
            # interior copy img -> xp[7:7+H, 7:7+W]
            nc.sync.dma_start(
                out=bass.AP(tensor=xp_ap.tensor, offset=xp_ap.offset + 7 * XPC + 7,
                            ap=[[XPC, H], [1, W]]),
                in_=bass.AP(tensor=x_ap.tensor, offset=x_ap.offset,
                            ap=[[W, H], [1, W]]))
            # uniform loads: partition p = cb*16 + rc gets xp rows
            # [rc*64, rc*64+78), cols [cb*128, cb*128+142)
            for cb in range(NCB):
                p0 = cb * NRC
                nc.sync.dma_start(
                    out=view(OFF_X, 0, XR, 0, PITCH, p0, p0 + NRC),
                    in_=bass.AP(tensor=xp_ap.tensor,
                                offset=xp_ap.offset + cb * B,
                                ap=[[A * XPC, NRC], [XPC, XR], [1, PITCH]]))

            # Fan-in: one tiny read per 32-partition group (engine APs must
            # start on 32-partition bounds) so no later instruction needs
            # more than a couple of semaphore waits; each group depends on
            # exactly 2 load DMAs.
            for g in range(4):
                p0 = g * 32
                nc.vector.tensor_copy(
                    view(OFF_TOUCH, 0, 1, 0, PITCH, p0, p0 + 32),
                    view(OFF_X, 40, 41, 0, PITCH, p0, p0 + 32))

            # ---------------- per-slab compute ----------------
            for s in range(NS):
                s0 = s * S

                # XSQ = X^2 over rows [s0-7, s0+15), all cols.  On DVE (not
                # ACT) so the scalar engine never reads DMA-written X: ACT
                # has too few sync-wait slots for the 8 DMA-queue sems.
                xwide = xv(s0 - 7, s0 + S + 7, -7, B + 7)
                nc.vector.tensor_mul(
                    sv(OFF_XSQ, s0, s0 - 7, s0 + S + 7, -7, B + 7),
                    xwide, xwide)

                # region tables per fwd dir: (y0,y1,x0,x1) for F, S2, S1
                regs = {}
                for nm, (uy, ux, _i) in FWD.items():
                    if nm == "a":
                        rf = (s0, s0 + S, -7, B)
                    elif nm == "b":
                        rf = (s0 - 7, s0 + S, 0, B)
                    elif nm == "c":
                        rf = (s0 - 7, s0 + S, -7, B)
                    else:
                        rf = (s0 - 7, s0 + S, 0, B + 7)

                    def ext(r, k):
                        y0, y1, x0, x1 = r
                        return (y0 + min(k * uy, 0), y1 + max(k * uy, 0),
                                x0 + min(k * ux, 0), x1 + max(k * ux, 0))
                    r2 = ext(rf, 4)
                    r1 = ext(r2, 2)
                    regs[nm] = (rf, r2, r1)

                def sh(r, dy, dx):
                    return (r[0] + dy, r[1] + dy, r[2] + dx, r[3] + dx)

                def doubling(nm, src_view, dst_off):
                    uy, ux, _ = FWD[nm]
                    rf, r2, r1 = regs[nm]
                    nc.vector.tensor_add(
                        sv(OFF_S1, s0, *r1), src_view(*r1), src_view(*sh(r1, uy, ux)))
                    nc.vector.tensor_add(
                        sv(OFF_S2, s0, *r2), sv(OFF_S1, s0, *r2),
                        sv(OFF_S1, s0, *sh(r2, 2 * uy, 2 * ux)))
                    nc.vector.tensor_add(
                        sv(dst_off, s0, *rf), sv(OFF_S2, s0, *rf),
                        sv(OFF_S2, s0, *sh(rf, 4 * uy, 4 * ux)))

                def xsqv(y0, y1, x0, x1):
                    return sv(OFF_XSQ, s0, y0, y1, x0, x1)

                # x1 sums for all 4 fwd dirs
                for nm, (uy, ux, i) in FWD.items():
                    doubling(nm, xv, OFF_F + i * (SZ_F + PAD))
                # sq_d = x1_d^2 on scalar engine, into the metric buffer
                for nm, (uy, ux, i) in FWD.items():
                    rf = regs[nm][0]
                    nc.scalar.square(sv(OFF_M + i * (SZ_F + PAD), s0, *rf),
                                     sv(OFF_F + i * (SZ_F + PAD), s0, *rf))
                # y2 sums + metric, interleaved (2 rotating y2 slots)
                for j, (nm, (uy, ux, i)) in enumerate(FWD.items()):
                    y2off = OFF_Y2 + (j % 2) * (SZ_F + PAD)
                    doubling(nm, xsqv, y2off)
                    rf = regs[nm][0]
                    moff = OFF_M + i * (SZ_F + PAD)
                    # m = sq*0.125 - y2   (== y1 - y2; maximize)
                    nc.vector.scalar_tensor_tensor(
                        out=sv(moff, s0, *rf), in0=sv(moff, s0, *rf), scalar=0.125,
                        in1=sv(y2off, s0, *rf),
                        op0=mybir.AluOpType.mult, op1=mybir.AluOpType.subtract)

                # ---------------- select chain (reference dir order) ----------------
                def mview(nm, dy, dx):
                    i = FWD[nm][2]
                    return sv(OFF_M + i * (SZ_F + PAD), s0,
                              s0 + dy, s0 + S + dy, dx, B + dx)

                def x1view(nm, dy, dx):
                    i = FWD[nm][2]
                    return sv(OFF_F + i * (SZ_F + PAD), s0,
                              s0 + dy, s0 + S + dy, dx, B + dx)

                bm = o3(OFF_BM)
                bx = o3(OFF_BX)
                lt = lt8[:, 0:S * PITCH].rearrange(
                    "p (r c) -> p r c", c=PITCH)[:, :, 0:B]
                nm0, dy0, dx0 = CHAIN[0]
                nc.vector.tensor_copy(bm, mview(nm0, dy0, dx0))
                nc.vector.tensor_copy(bx, x1view(nm0, dy0, dx0))
                for nm, dy, dx in CHAIN[1:]:
                    mv = mview(nm, dy, dx)
                    nc.vector.tensor_tensor(lt, mv, bm, mybir.AluOpType.is_gt)
                    nc.vector.copy_predicated(bm, lt, mv)
                    nc.vector.copy_predicated(bx, lt, x1view(nm, dy, dx))

                # out = bx / 8 — split per 32-partition group so each ACT op
                # waits on at most 2 out-DMA queue sems (WAR on the slot)
                oslot = OFF_OUT + (s % 2) * (SZ_O + PAD)
                for g in range(4):
                    nc.scalar.mul(o3(oslot, g * 32, g * 32 + 32),
                                  o3(OFF_BX, g * 32, g * 32 + 32), 0.125)

                # store: per col-block DMA
                for cb in range(NCB):
                    p0 = cb * NRC
                    nc.gpsimd.dma_start(
                        out=bass.AP(tensor=y_ap.tensor,
                                    offset=y_ap.offset + s0 * W + cb * B,
                                    ap=[[A * W, NRC], [W, S], [1, B]]),
                        in_=o3(oslot, p0, p0 + NRC))
    nc.compile()
    return nc


_nc_cache = []


def _get_nc():
    if not _nc_cache:
        _nc_cache.append(_build())
    return _nc_cache[0]


def kernel(x, weight=None, _want_results=False, **_ignored):
    x = np.ascontiguousarray(np.asarray(x), dtype=np.float32)
    n = x.shape[0]
    assert x.shape == (n, 1, H, W), x.shape
    nc = _get_nc()
    in_maps = [{"x": np.ascontiguousarray(x[i, 0])} for i in range(n)]
    res = run_bass_kernel_spmd(nc, in_maps, core_ids=list(range(n)))
    out = np.stack([r["y"] for r in res.results])[:, None]
    if _want_results:
        return out, res
    return out


if __name__ == "__main__":
    rng = np.random.default_rng(0)
    x = rng.standard_normal((8, 1, H, W)).astype(np.float32)
    y = kernel(x)
    print("ran; out shape", y.shape, "mean", y.mean())
